# revision 17
# baseline (speedup 1.0000x reference)
"""Distributed Trainium2 Bass kernel for the fused attention layer.

Problem (hardcoded):
    B=2, S=2048, D=1024, H=16, HD=64.
    out = softmax((q@Wq+bq) @ (k@Wk+bk)^T / sqrt(HD)) @ (v@Wv+bv), per (b, h).

Sharding: 8 cores, core c -> batch b = c//4, head group hg = c%4 (4 heads).
Each core computes its 4 heads fully independently (no collectives) and
writes a transposed [256, S] bf16 slab; the host reassembles [B, S, D].

Per-core dataflow (matmul inputs bf16, PSUM accumulation f32):
  QhatT [2*128, S]  = (Wq_cols^T @ q_b^T) + bq   (heads on partitions)
  KhatT [2*128, S]  = (Wk_cols^T @ k_b^T) + bk
  Vaug  [S, 4*65]   = (v_b @ Wv_cols) + bv, with a ones column per head
  per head h:
    E[k, q]    = exp((KhatT_h-slice^T x QhatT_h) / 8)   (scores transposed,
                 ScalarE exp straight from PSUM, no max subtraction)
    U[65, q]   = sum_kt Vaug_h[kt]^T @ E[kt]  (row 64 = softmax denominator)
    outT[d, q] = U[0:64, q] * (1/U[64, q])    (denom broadcast via DMA)
Heads are software-pipelined: attn-V of head h-1 interleaves with
scores+exp of head h so ScalarE (the exp bottleneck) never idles.
"""

import sys
import os

for _p in ("/opt/trn_rl_repo",):
    if os.path.isdir(_p) and _p not in sys.path:
        sys.path.append(_p)

import numpy as np
import ml_dtypes

import concourse.bacc as bacc
import concourse.mybir as mybir
from concourse import tile
from concourse.bass_utils import run_bass_kernel_spmd
from concourse import dve_ops as _dvo
from concourse.dve_spec import (
    Spec as _Spec, Src0 as _S0, C0 as _C0, C1 as _C1, C2 as _C2,
    sq as _sq, lower as _lower,
)
from concourse.dve_uop import DveOpSpec as _DveOpSpec

# exp(x/8) ~= P(x/512)^64, P the cubic Taylor of e^z. Two 1-pass custom
# DVE ops (~1 elem/cycle/lane each); max rel err ~3e-4 for |x/8| <= 6.5.
_EXP_SC = 1.0 / 512.0
_EXP_ABC = (_EXP_SC ** 3 / 6.0, _EXP_SC ** 2 / 2.0, _EXP_SC)


def _register_exp_ops():
    if "EXP_POLY_ANT" in _dvo._SUB_OPCODE_FOR_NAME:
        by = {o.name: o for o in _dvo.OPS}
        return by["EXP_POLY_ANT"], by["EXP_SQ6_ANT"]

    def ref1(in0, in1, s0, s1, imm2):
        return (((in0 * s0 + s1) * in0 + imm2) * in0).astype(np.float32)

    def ref2(in0, in1, s0, s1, imm2):
        v = (in0 + s0).astype(np.float32)
        for _ in range(6):
            v = (v * v).astype(np.float32)
        return v

    body1 = ((_S0 * _C0 + _C1) * _S0 + _C2) * _S0
    body2 = _S0 + _C0
    for _ in range(6):
        body2 = _sq(body2)
    made = []
    for name, body, ref in (("EXP_POLY_ANT", body1, ref1),
                            ("EXP_SQ6_ANT", body2, ref2)):
        spec = _Spec(body=body, reference=ref)
        row = _dvo._CUSTOM_DVE_ROW_BASE + len(_dvo.OPS)
        shas = {}
        for ver in ("v3", "v4"):
            s = _DveOpSpec(name=name, opcode=row, uops=_lower(spec, ver=ver),
                           rd1_en=False)
            shas[ver] = s.sha(ver)
        op = _dvo.DveOp(name, spec, subdim=False, uops_sha=shas)
        _dvo.OPS.append(op)
        _dvo.CUSTOM_DVE_SPECS[name] = spec
        _dvo._SUB_OPCODE_FOR_NAME[name] = row
        made.append(op)
    return made


_EXP1_OP, _EXP2_OP = _register_exp_ops()


BF16 = ml_dtypes.bfloat16
N_CORES = 8
B, S, D, H = 2, 2048, 1024, 16
HD = D // H            # 64
HL = 4                 # local heads per core
DH = HL * HD           # 256 local out channels
KT = D // 128          # 8 contraction tiles for projections
ST = S // 128          # 16 sequence tiles

_CACHED = {}


def _build():
    f32 = mybir.dt.float32
    bf = mybir.dt.bfloat16
    EXP = mybir.ActivationFunctionType.Exp
    nc = bacc.Bacc(
        "TRN2", target_bir_lowering=False, debug=False, num_devices=N_CORES
    )

    qT_d = nc.dram_tensor("qT", [D, S], bf, kind="ExternalInput")
    kT_d = nc.dram_tensor("kT", [D, S], bf, kind="ExternalInput")
    vT_d = nc.dram_tensor("vT", [D, S], bf, kind="ExternalInput")
    wq_d = nc.dram_tensor("wq", [D, DH], bf, kind="ExternalInput")
    wk_d = nc.dram_tensor("wk", [D, DH], bf, kind="ExternalInput")
    wv_d = nc.dram_tensor("wv", [D, DH], bf, kind="ExternalInput")
    bq_d = nc.dram_tensor("bq", [128, 2], f32, kind="ExternalInput")
    bk_d = nc.dram_tensor("bk", [128, 2], f32, kind="ExternalInput")
    bv_d = nc.dram_tensor("bv", [128, DH], f32, kind="ExternalInput")
    out_d = nc.dram_tensor("out", [DH, S], bf, kind="ExternalOutput")

    with tile.TileContext(nc) as tc:
        with (
            tc.tile_pool(name="big", bufs=24) as big,
            tc.tile_pool(name="wp", bufs=1) as wp,
            tc.tile_pool(name="qk", bufs=1) as qk,
            tc.tile_pool(name="vp", bufs=1) as vp,
            tc.tile_pool(name="np_", bufs=1) as npl,
            tc.tile_pool(name="outp", bufs=1) as outp,
            tc.tile_pool(name="dv", bufs=2) as dvp,
            tc.tile_pool(name="ps", bufs=1, space="PSUM") as ps,
            tc.tile_pool(name="aps", bufs=1, space="PSUM") as aps,
        ):
            # ---- loads (order matters: Q-projection deps first) ----
            def load_rows(dram, n_tiles, width, pool, eng):
                ts = []
                for i in range(n_tiles):
                    t = (pool.tile([128, width], bf, tag="big",
                                   name=f"{dram.name}t{i}")
                         if pool is big else
                         pool.tile([128, width], bf, tag=f"{dram.name}{i}",
                                   name=f"{dram.name}t{i}"))
                    eng.dma_start(t[:], dram[i * 128:(i + 1) * 128, :])
                    ts.append(t)
                return ts

            warm = wp.tile([128, 512], bf, tag="warm", name="warm_t")
            nc.gpsimd.memset(warm[:], 0.0)
            for w_i in range(10):
                wps = ps.tile([128, 1024], f32, tag="psx", name="warm_ps")
                nc.tensor.matmul(wps[:, 0:512], warm[:, 0:128], warm[:],
                                 start=True, stop=True)

            wq_t = load_rows(wq_d, KT, DH, wp, nc.gpsimd)
            bq_t = wp.tile([128, 2], f32, tag="bq", name="bq_t")
            nc.gpsimd.dma_start(bq_t[:], bq_d[:])
            qT_t = load_rows(qT_d, KT, S, big, nc.sync)
            wk_t = load_rows(wk_d, KT, DH, wp, nc.gpsimd)
            bk_t = wp.tile([128, 2], f32, tag="bk", name="bk_t")
            nc.gpsimd.dma_start(bk_t[:], bk_d[:])
            kT_t = load_rows(kT_d, KT, S, big, nc.sync)
            wv_t = load_rows(wv_d, KT, DH, wp, nc.gpsimd)
            bv_t = wp.tile([128, DH], f32, tag="bv", name="bv_t")
            nc.gpsimd.dma_start(bv_t[:], bv_d[:])
            vT_t = load_rows(vT_d, KT, S, big, nc.sync)

            qhat = [qk.tile([128, S], bf, tag=f"qh{mp}", name=f"qhat{mp}")
                    for mp in range(2)]
            khat = [qk.tile([128, S], bf, tag=f"kh{mp}", name=f"khat{mp}")
                    for mp in range(2)]
            vaug = [None] * ST
            texp = {}

            def qk_chain(src_t, w_t, b_t, dst, mp, nch):
                pst = ps.tile([128, 1024], f32, tag="psx", name="proj_ps")
                for kt in range(KT):
                    nc.tensor.matmul(
                        pst[:, 0:512],
                        w_t[kt][:, mp * 128:(mp + 1) * 128],
                        src_t[kt][:, nch * 512:(nch + 1) * 512],
                        start=(kt == 0), stop=(kt == KT - 1),
                    )
                nc.vector.tensor_scalar_add(
                    dst[mp][:, nch * 512:(nch + 1) * 512],
                    pst[:, 0:512], b_t[:, mp:mp + 1],
                )

            def v_chain(st):
                pst = ps.tile([128, 1024], f32, tag="psx", name="vproj_ps")
                for kt in range(KT):
                    nc.tensor.matmul(
                        pst[:, 0:DH],
                        vT_t[kt][:, st * 128:(st + 1) * 128],
                        wv_t[kt][:],
                        start=(kt == 0), stop=(kt == KT - 1),
                    )
                va = vp.tile([128, HL * 65], bf, tag=f"va{st}",
                             name=f"vaug{st}")
                nc.vector.memset(va[:], 1.0)
                for h in range(HL):
                    nc.vector.tensor_add(
                        va[:, h * 65:h * 65 + 64],
                        pst[:, h * 64:(h + 1) * 64],
                        bv_t[:, h * 64:(h + 1) * 64],
                    )
                vaug[st] = va

            def scores_pair(h, kp, n_dve=0):
                # kt = 2*kp uses khat/qhat rows [hb:hb+64] (row group A);
                # kt = 2*kp+1 uses ksh/qsh rows [64-hb:...] (row group B);
                # adjacent emission -> concurrent PE row-group execution.
                mp, hb = h // 2, (h % 2) * 64
                sb = 64 - hb
                txs = []
                for kt in (2 * kp, 2 * kp + 1):
                    txs.append(big.tile([128, S], bf, tag="big",
                                        name=f"texp_{h}_{kt}"))
                    texp[(h, kt)] = txs[-1]
                for half in range(2):
                    spts = []
                    for i, kt in enumerate((2 * kp, 2 * kp + 1)):
                        spts.append(ps.tile([128, 1024], f32,
                                            tag=("pse", "pso")[i],
                                            name=f"score_ps{i}"))
                    for qch in range(2):
                        qlo = half * 1024 + qch * 512
                        for i, kt in enumerate((2 * kp, 2 * kp + 1)):
                            kh = khat[mp] if i == 0 else ksh[mp]
                            qh = qhat[mp] if i == 0 else qsh[mp]
                            b0 = hb if i == 0 else sb
                            nc.tensor.matmul(
                                spts[i][:, qch * 512:(qch + 1) * 512],
                                kh[b0:b0 + 64, kt * 128:(kt + 1) * 128],
                                qh[b0:b0 + 64, qlo:qlo + 512],
                                start=True, stop=True,
                            )
                    for i in range(2):
                        txs_ap = txs[i][:, half * 1024:(half + 1) * 1024]
                        if (half + i) % 2 < n_dve:
                            tmp = dvp.tile([128, 1024], f32, tag="dvtmp",
                                           name=f"dvt_{h}_{kp}_{half}_{i}")
                            a, b, c = _EXP_ABC
                            nc.vector._custom_dve(
                                _EXP1_OP, out=tmp[:], in0=spts[i][:],
                                s0=a, s1=b, imm2=c)
                            nc.vector._custom_dve(
                                _EXP2_OP, out=txs_ap, in0=tmp[:], s0=1.0)
                        else:
                            nc.scalar.activation(txs_ap, spts[i][:],
                                                 EXP, scale=0.125)

            def attnv_substep(h, ktp, c, apt):
                # chains for nch in {2c, 2c+1}; ktp advances 0..15
                for j in range(2):
                    nch = 2 * c + j
                    nc.tensor.matmul(
                        apt[j][0:65, :],
                        vaug[ktp][:, h * 65:h * 65 + 65],
                        texp[(h, ktp)][:, nch * 512:(nch + 1) * 512],
                        start=(ktp == 0), stop=(ktp == ST - 1),
                    )

            usb_t = {}

            def evac(h, tile_, nch):
                u = npl.tile([96, 512], f32, tag=f"usb{nch}",
                             name=f"usb_{h}_{nch}")
                nc.vector.tensor_copy(u[:], tile_[0:96, :])
                usb_t[(h, nch)] = u

            def finish_norm(h):
                bc = norm_t[h][2]
                ot = outp.tile([64, S], bf, tag="ot", name=f"outT{h}")
                for nch in range(4):
                    nc.vector.tensor_mul(
                        ot[:, nch * 512:(nch + 1) * 512],
                        usb_t[(h, nch)][0:64, :],
                        bc[:, nch * 512:(nch + 1) * 512],
                    )
                nc.sync.dma_start(out_d[h * 64:(h + 1) * 64, :], ot[:])

            # ---- schedule ----
            for nch in range(4):
                qk_chain(qT_t, wq_t, bq_t, qhat, 0, nch)
            for nch in range(4):
                qk_chain(kT_t, wk_t, bk_t, khat, 0, nch)
            for nch in range(4):
                qk_chain(qT_t, wq_t, bq_t, qhat, 1, nch)
            for nch in range(4):
                qk_chain(kT_t, wk_t, bk_t, khat, 1, nch)
            # half-swapped shadows: rows 0-63 <-> 64-127, so adjacent k-tiles
            # of one head can run in both PE row groups concurrently
            qsh = [qk.tile([128, S], bf, tag=f"qs{mp}", name=f"qsh{mp}")
                   for mp in range(2)]
            ksh = [qk.tile([128, S], bf, tag=f"ks{mp}", name=f"ksh{mp}")
                   for mp in range(2)]
            for mp in range(2):
                for sh, hat in ((qsh, qhat), (ksh, khat)):
                    nc.gpsimd.dma_start(sh[mp][0:64, :], hat[mp][64:128, :])
                    nc.gpsimd.dma_start(sh[mp][64:128, :], hat[mp][0:64, :])

            def alloc_apt(h, j):
                return aps.tile([96, 512], f32, tag=f"at{j % 2}",
                                name=f"attn_ps_{h}_{j}")

            norm_t = {}

            def half_norm(h, c):
                # denominator -> reciprocal -> partition broadcast for the
                # q-range covered by sweep c (nch 2c, 2c+1)
                if c == 0:
                    norm_t[h] = (
                        npl.tile([1, S], f32, tag="denrow", name=f"den{h}"),
                        npl.tile([1, S], f32, tag="rec0", name=f"rec0_{h}"),
                        npl.tile([64, S], f32, tag="bc", name=f"bc{h}"),
                    )
                denrow, rec0, bc = norm_t[h]
                for nch in (2 * c, 2 * c + 1):
                    nc.sync.dma_start(
                        denrow[0:1, nch * 512:(nch + 1) * 512],
                        usb_t[(h, nch)][64:65, :],
                    )
                sl = slice(c * 1024, (c + 1) * 1024)
                nc.vector.reciprocal_approx_fast(rec0[0:1, sl], denrow[0:1, sl])
                nc.gpsimd.partition_broadcast(bc[:, sl], rec0[0:1, sl])

            def emit_b(h, kt, apt):
                # two B-substeps per A-step: nch chains {0,1} during kt 0-7,
                # {2,3} during kt 8-15 (only 2 attn PSUM banks live at once)
                if kt == 8:
                    evac(h, apt[0], 0)
                    evac(h, apt[1], 1)
                    half_norm(h, 0)
                    apt[0], apt[1] = alloc_apt(h, 2), alloc_apt(h, 3)
                c, base = (0, 0) if kt < 8 else (1, 8)
                for j in range(2):
                    attnv_substep(h, 2 * (kt - base) + j, c, apt)

            def finish_b(h, apt):
                evac(h, apt[0], 2)
                evac(h, apt[1], 3)
                half_norm(h, 1)
                finish_norm(h)

            apt_prev = None
            for h in range(HL):
                apt = [alloc_apt(h, 0), alloc_apt(h, 1)]
                dve_kps = ({2, 5} if h == 0 else {0, 2, 4, 6})
                for kp in range(ST // 2):
                    for sub in range(2):
                        if apt_prev is not None:
                            emit_b(h - 1, 2 * kp + sub, apt_prev)
                        if h == 0:
                            v_chain(2 * kp + sub)
                    scores_pair(h, kp, n_dve=1 if kp in dve_kps else 0)
                if apt_prev is not None:
                    finish_b(h - 1, apt_prev)
                apt_prev = apt
            for kt in range(ST):
                emit_b(HL - 1, kt, apt_prev)
            finish_b(HL - 1, apt_prev)

    nc.compile()
    return nc


def _prep_in_maps(q, k, v, Wq, bq, Wk, bk, Wv, bv):
    qT = [np.ascontiguousarray(q[b].T.astype(BF16)) for b in range(B)]
    kT = [np.ascontiguousarray(k[b].T.astype(BF16)) for b in range(B)]
    vT = [np.ascontiguousarray(v[b].T.astype(BF16)) for b in range(B)]
    in_maps = []
    for c in range(N_CORES):
        b, hg = c // 4, c % 4
        cols = slice(hg * DH, (hg + 1) * DH)
        in_maps.append({
            "qT": qT[b],
            "kT": kT[b],
            "vT": vT[b],
            "wq": np.ascontiguousarray(Wq[:, cols].astype(BF16)),
            "wk": np.ascontiguousarray(Wk[:, cols].astype(BF16)),
            "wv": np.ascontiguousarray(Wv[:, cols].astype(BF16)),
            "bq": np.ascontiguousarray(
                bq[cols].astype(np.float32).reshape(2, 128).T),
            "bk": np.ascontiguousarray(
                bk[cols].astype(np.float32).reshape(2, 128).T),
            "bv": np.ascontiguousarray(
                np.tile(bv[cols].astype(np.float32), (128, 1))),
        })
    return in_maps


def kernel(q, k, v, Wq, bq, Wk, bk, Wv, bv, _trace=False, _trace_cores=None):
    q, k, v = (np.asarray(x, np.float32) for x in (q, k, v))
    Wq, Wk, Wv = (np.asarray(x, np.float32) for x in (Wq, Wk, Wv))
    bq, bk, bv = (np.asarray(x, np.float32) for x in (bq, bk, bv))

    if "nc" not in _CACHED:
        _CACHED["nc"] = _build()
    nc = _CACHED["nc"]

    in_maps = _prep_in_maps(q, k, v, Wq, bq, Wk, bk, Wv, bv)
    res = run_bass_kernel_spmd(
        nc, in_maps, core_ids=list(range(N_CORES)),
        trace=_trace, trace_cores=_trace_cores,
    )
    _CACHED["last_result"] = res

    out = np.empty((B, S, D), np.float32)
    for c in range(N_CORES):
        b, hg = c // 4, c % 4
        out[b, :, hg * DH:(hg + 1) * DH] = \
            res.results[c]["out"].T.astype(np.float32)
    return out


# revision 18
# speedup vs baseline: 1.1982x; 1.1982x over previous
"""Distributed Trainium2 Bass kernel for the fused attention layer.

Problem (hardcoded):
    B=2, S=2048, D=1024, H=16, HD=64.
    out = softmax((q@Wq+bq) @ (k@Wk+bk)^T / sqrt(HD)) @ (v@Wv+bv), per (b, h).

Sharding: 8 cores, core c -> batch b = c//4, head group hg = c%4 (4 heads).
Each core computes its 4 heads fully independently (no collectives) and
writes a transposed [256, S] bf16 slab; the host reassembles [B, S, D].

Per-core dataflow (matmul inputs bf16, PSUM accumulation f32):
  QhatT [2*128, S]  = (Wq_cols^T @ q_b^T) + bq   (heads on partitions)
  KhatT [2*128, S]  = (Wk_cols^T @ k_b^T) + bk
  Vaug  [S, 4*65]   = (v_b @ Wv_cols) + bv, with a ones column per head
  per head h:
    E[k, q]    = exp((KhatT_h-slice^T x QhatT_h) / 8)   (scores transposed,
                 ScalarE exp straight from PSUM, no max subtraction)
    U[65, q]   = sum_kt Vaug_h[kt]^T @ E[kt]  (row 64 = softmax denominator)
    outT[d, q] = U[0:64, q] * (1/U[64, q])    (denom broadcast via DMA)
Heads are software-pipelined: attn-V of head h-1 interleaves with
scores+exp of head h so ScalarE (the exp bottleneck) never idles.
"""

import sys
import os

for _p in ("/opt/trn_rl_repo",):
    if os.path.isdir(_p) and _p not in sys.path:
        sys.path.append(_p)

import numpy as np
import ml_dtypes

import concourse.bacc as bacc
import concourse.mybir as mybir
from concourse import tile
from concourse.bass_utils import run_bass_kernel_spmd
from concourse import dve_ops as _dvo
from concourse.dve_spec import (
    Spec as _Spec, Src0 as _S0, C0 as _C0, C1 as _C1, C2 as _C2,
    sq as _sq, lower as _lower,
)
from concourse.dve_uop import DveOpSpec as _DveOpSpec

# exp(x/8) ~= P(x/512)^64, P the cubic Taylor of e^z. Two 1-pass custom
# DVE ops (~1 elem/cycle/lane each); max rel err ~3e-4 for |x/8| <= 6.5.
_EXP_SC = 1.0 / 512.0
_EXP_ABC = (_EXP_SC ** 3 / 6.0, _EXP_SC ** 2 / 2.0, _EXP_SC)


def _register_exp_ops():
    if "EXP_POLY_ANT" in _dvo._SUB_OPCODE_FOR_NAME:
        by = {o.name: o for o in _dvo.OPS}
        return by["EXP_POLY_ANT"], by["EXP_SQ6_ANT"]

    def ref1(in0, in1, s0, s1, imm2):
        return (((in0 * s0 + s1) * in0 + imm2) * in0).astype(np.float32)

    def ref2(in0, in1, s0, s1, imm2):
        v = (in0 + s0).astype(np.float32)
        for _ in range(6):
            v = (v * v).astype(np.float32)
        return v

    body1 = ((_S0 * _C0 + _C1) * _S0 + _C2) * _S0
    body2 = _S0 + _C0
    for _ in range(6):
        body2 = _sq(body2)
    made = []
    for name, body, ref in (("EXP_POLY_ANT", body1, ref1),
                            ("EXP_SQ6_ANT", body2, ref2)):
        spec = _Spec(body=body, reference=ref)
        row = _dvo._CUSTOM_DVE_ROW_BASE + len(_dvo.OPS)
        shas = {}
        for ver in ("v3", "v4"):
            s = _DveOpSpec(name=name, opcode=row, uops=_lower(spec, ver=ver),
                           rd1_en=False)
            shas[ver] = s.sha(ver)
        op = _dvo.DveOp(name, spec, subdim=False, uops_sha=shas)
        _dvo.OPS.append(op)
        _dvo.CUSTOM_DVE_SPECS[name] = spec
        _dvo._SUB_OPCODE_FOR_NAME[name] = row
        made.append(op)
    return made


_EXP1_OP, _EXP2_OP = _register_exp_ops()


BF16 = ml_dtypes.bfloat16
N_CORES = 8
B, S, D, H = 2, 2048, 1024, 16
HD = D // H            # 64
HL = 4                 # local heads per core
DH = HL * HD           # 256 local out channels
KT = D // 128          # 8 contraction tiles for projections
ST = S // 128          # 16 sequence tiles

_CACHED = {}


def _build():
    f32 = mybir.dt.float32
    bf = mybir.dt.bfloat16
    EXP = mybir.ActivationFunctionType.Exp
    nc = bacc.Bacc(
        "TRN2", target_bir_lowering=False, debug=False, num_devices=N_CORES
    )

    qT_d = nc.dram_tensor("qT", [D, S], bf, kind="ExternalInput")
    kT_d = nc.dram_tensor("kT", [D, S], bf, kind="ExternalInput")
    vT_d = nc.dram_tensor("vT", [D, S], bf, kind="ExternalInput")
    wq_d = nc.dram_tensor("wq", [D, DH], bf, kind="ExternalInput")
    wk_d = nc.dram_tensor("wk", [D, DH], bf, kind="ExternalInput")
    wv_d = nc.dram_tensor("wv", [D, DH], bf, kind="ExternalInput")
    bq_d = nc.dram_tensor("bq", [128, 2], f32, kind="ExternalInput")
    bk_d = nc.dram_tensor("bk", [128, 2], f32, kind="ExternalInput")
    bv_d = nc.dram_tensor("bv", [128, DH], f32, kind="ExternalInput")
    out_d = nc.dram_tensor("out", [DH, S], bf, kind="ExternalOutput")

    with tile.TileContext(nc) as tc:
        with (
            tc.tile_pool(name="big", bufs=24) as big,
            tc.tile_pool(name="wp", bufs=1) as wp,
            tc.tile_pool(name="qk", bufs=1) as qk,
            tc.tile_pool(name="vp", bufs=1) as vp,
            tc.tile_pool(name="np_", bufs=1) as npl,
            tc.tile_pool(name="outp", bufs=1) as outp,
            tc.tile_pool(name="dv", bufs=2) as dvp,
            tc.tile_pool(name="ps", bufs=1, space="PSUM") as ps,
            tc.tile_pool(name="aps", bufs=1, space="PSUM") as aps,
        ):
            # ---- loads (order matters: Q-projection deps first) ----
            def load_rows(dram, n_tiles, width, pool, eng):
                ts = []
                for i in range(n_tiles):
                    t = (pool.tile([128, width], bf, tag="big",
                                   name=f"{dram.name}t{i}")
                         if pool is big else
                         pool.tile([128, width], bf, tag=f"{dram.name}{i}",
                                   name=f"{dram.name}t{i}"))
                    eng.dma_start(t[:], dram[i * 128:(i + 1) * 128, :])
                    ts.append(t)
                return ts

            warm = wp.tile([128, 512], bf, tag="warm", name="warm_t")
            nc.gpsimd.memset(warm[:], 0.0)
            for w_i in range(10):
                wps = ps.tile([128, 512], f32, tag="psx", name="warm_ps",
                              bufs=2)
                nc.tensor.matmul(wps[:], warm[:, 0:128], warm[:],
                                 start=True, stop=True)

            wq_t = load_rows(wq_d, KT, DH, wp, nc.gpsimd)
            bq_t = wp.tile([128, 2], f32, tag="bq", name="bq_t")
            nc.gpsimd.dma_start(bq_t[:], bq_d[:])
            qT_t = load_rows(qT_d, KT, S, big, nc.sync)
            wk_t = load_rows(wk_d, KT, DH, wp, nc.gpsimd)
            bk_t = wp.tile([128, 2], f32, tag="bk", name="bk_t")
            nc.gpsimd.dma_start(bk_t[:], bk_d[:])
            kT_t = load_rows(kT_d, KT, S, big, nc.sync)
            wv_t = load_rows(wv_d, KT, DH, wp, nc.gpsimd)
            bv_t = wp.tile([128, DH], f32, tag="bv", name="bv_t")
            nc.gpsimd.dma_start(bv_t[:], bv_d[:])
            vT_t = load_rows(vT_d, KT, S, big, nc.sync)

            qhat = [qk.tile([128, S], bf, tag=f"qh{mp}", name=f"qhat{mp}")
                    for mp in range(2)]
            khat = [qk.tile([128, S], bf, tag=f"kh{mp}", name=f"khat{mp}")
                    for mp in range(2)]
            vaug = [None] * ST
            texp = {}

            def qk_chain(src_t, w_t, b_t, dst, mp, nch):
                pst = ps.tile([128, 512], f32, tag="psx", name="proj_ps",
                              bufs=2)
                for kt in range(KT):
                    nc.tensor.matmul(
                        pst[:, 0:512],
                        w_t[kt][:, mp * 128:(mp + 1) * 128],
                        src_t[kt][:, nch * 512:(nch + 1) * 512],
                        start=(kt == 0), stop=(kt == KT - 1),
                    )
                nc.vector.tensor_scalar_add(
                    dst[mp][:, nch * 512:(nch + 1) * 512],
                    pst[:, 0:512], b_t[:, mp:mp + 1],
                )

            def v_chain(st):
                pst = ps.tile([128, 512], f32, tag="psx", name="vproj_ps",
                              bufs=2)
                for kt in range(KT):
                    nc.tensor.matmul(
                        pst[:, 0:DH],
                        vT_t[kt][:, st * 128:(st + 1) * 128],
                        wv_t[kt][:],
                        start=(kt == 0), stop=(kt == KT - 1),
                    )
                va = vp.tile([128, HL * 65], bf, tag=f"va{st}",
                             name=f"vaug{st}")
                nc.vector.memset(va[:], 1.0)
                for h in range(HL):
                    nc.vector.tensor_add(
                        va[:, h * 65:h * 65 + 64],
                        pst[:, h * 64:(h + 1) * 64],
                        bv_t[:, h * 64:(h + 1) * 64],
                    )
                vaug[st] = va

            def scores_pair(h, kp, n_dve=0):
                # kt = 2*kp uses khat/qhat rows [hb:hb+64] (row group A);
                # kt = 2*kp+1 uses ksh/qsh rows [64-hb:...] (row group B);
                # adjacent emission -> concurrent PE row-group execution.
                mp, hb = h // 2, (h % 2) * 64
                sb = 64 - hb
                txs = []
                for kt in (2 * kp, 2 * kp + 1):
                    txs.append(big.tile([128, S], bf, tag="big",
                                        name=f"texp_{h}_{kt}"))
                    texp[(h, kt)] = txs[-1]
                for half in range(2):
                    spts = []
                    for i, kt in enumerate((2 * kp, 2 * kp + 1)):
                        spts.append(ps.tile([128, 1024], f32,
                                            tag=("pse", "pso")[i],
                                            name=f"score_ps{i}"))
                    for qch in range(2):
                        qlo = half * 1024 + qch * 512
                        for i, kt in enumerate((2 * kp, 2 * kp + 1)):
                            kh = khat[mp] if i == 0 else ksh[mp]
                            qh = qhat[mp] if i == 0 else qsh[mp]
                            b0 = hb if i == 0 else sb
                            nc.tensor.matmul(
                                spts[i][:, qch * 512:(qch + 1) * 512],
                                kh[b0:b0 + 64, kt * 128:(kt + 1) * 128],
                                qh[b0:b0 + 64, qlo:qlo + 512],
                                start=True, stop=True,
                            )
                    for i in range(2):
                        txs_ap = txs[i][:, half * 1024:(half + 1) * 1024]
                        if (half + i) % 2 < n_dve:
                            tmp = dvp.tile([128, 1024], f32, tag="dvtmp",
                                           name=f"dvt_{h}_{kp}_{half}_{i}")
                            a, b, c = _EXP_ABC
                            nc.vector._custom_dve(
                                _EXP1_OP, out=tmp[:], in0=spts[i][:],
                                s0=a, s1=b, imm2=c)
                            nc.vector._custom_dve(
                                _EXP2_OP, out=txs_ap, in0=tmp[:], s0=1.0)
                        else:
                            nc.scalar.activation(txs_ap, spts[i][:],
                                                 EXP, scale=0.125)

            def attnv_substep(h, ktp, c, apt):
                # chains for nch in {2c, 2c+1}; ktp advances 0..15
                for j in range(2):
                    nch = 2 * c + j
                    nc.tensor.matmul(
                        apt[j][0:65, :],
                        vaug[ktp][:, h * 65:h * 65 + 65],
                        texp[(h, ktp)][:, nch * 512:(nch + 1) * 512],
                        start=(ktp == 0), stop=(ktp == ST - 1),
                    )

            usb_t = {}

            def evac(h, tile_, nch):
                u = npl.tile([96, 512], f32, tag=f"usb{nch}",
                             name=f"usb_{h}_{nch}")
                nc.vector.tensor_copy(u[:], tile_[0:96, :])
                usb_t[(h, nch)] = u

            def finish_norm(h):
                bc = norm_t[h][2]
                ot = outp.tile([64, S], bf, tag="ot", name=f"outT{h}")
                for nch in range(4):
                    nc.vector.tensor_mul(
                        ot[:, nch * 512:(nch + 1) * 512],
                        usb_t[(h, nch)][0:64, :],
                        bc[:, nch * 512:(nch + 1) * 512],
                    )
                nc.sync.dma_start(out_d[h * 64:(h + 1) * 64, :], ot[:])

            # ---- schedule ----
            for nch in range(4):
                qk_chain(qT_t, wq_t, bq_t, qhat, 0, nch)
            for nch in range(4):
                qk_chain(kT_t, wk_t, bk_t, khat, 0, nch)
            for nch in range(4):
                qk_chain(qT_t, wq_t, bq_t, qhat, 1, nch)
            for nch in range(4):
                qk_chain(kT_t, wk_t, bk_t, khat, 1, nch)
            # half-swapped shadows: rows 0-63 <-> 64-127, so adjacent k-tiles
            # of one head can run in both PE row groups concurrently
            qsh = [qk.tile([128, S], bf, tag=f"qs{mp}", name=f"qsh{mp}")
                   for mp in range(2)]
            ksh = [qk.tile([128, S], bf, tag=f"ks{mp}", name=f"ksh{mp}")
                   for mp in range(2)]
            for mp in range(2):
                for sh, hat in ((qsh, qhat), (ksh, khat)):
                    nc.gpsimd.dma_start(sh[mp][0:64, :], hat[mp][64:128, :])
                    nc.gpsimd.dma_start(sh[mp][64:128, :], hat[mp][0:64, :])

            def alloc_apt(h, j):
                return aps.tile([96, 512], f32, tag=f"at{j % 2}",
                                name=f"attn_ps_{h}_{j}")

            norm_t = {}

            def half_norm(h, c):
                # denominator -> reciprocal -> partition broadcast for the
                # q-range covered by sweep c (nch 2c, 2c+1)
                if c == 0:
                    norm_t[h] = (
                        npl.tile([1, S], f32, tag="denrow", name=f"den{h}"),
                        npl.tile([1, S], f32, tag="rec0", name=f"rec0_{h}"),
                        npl.tile([64, S], f32, tag="bc", name=f"bc{h}"),
                    )
                denrow, rec0, bc = norm_t[h]
                for nch in (2 * c, 2 * c + 1):
                    nc.sync.dma_start(
                        denrow[0:1, nch * 512:(nch + 1) * 512],
                        usb_t[(h, nch)][64:65, :],
                    )
                sl = slice(c * 1024, (c + 1) * 1024)
                nc.vector.reciprocal_approx_fast(rec0[0:1, sl], denrow[0:1, sl])
                nc.gpsimd.partition_broadcast(bc[:, sl], rec0[0:1, sl])

            def emit_b(h, kt, apt):
                # two B-substeps per A-step: nch chains {0,1} during kt 0-7,
                # {2,3} during kt 8-15 (only 2 attn PSUM banks live at once)
                if kt == 8:
                    evac(h, apt[0], 0)
                    evac(h, apt[1], 1)
                    half_norm(h, 0)
                    apt[0], apt[1] = alloc_apt(h, 2), alloc_apt(h, 3)
                c, base = (0, 0) if kt < 8 else (1, 8)
                for j in range(2):
                    attnv_substep(h, 2 * (kt - base) + j, c, apt)

            def finish_b(h, apt):
                evac(h, apt[0], 2)
                evac(h, apt[1], 3)
                half_norm(h, 1)
                finish_norm(h)

            apt_prev = None
            for h in range(HL):
                apt = [alloc_apt(h, 0), alloc_apt(h, 1)]
                dve_kps = ({2, 5} if h == 0 else {0, 2, 4, 6})
                for kp in range(ST // 2):
                    for sub in range(2):
                        if apt_prev is not None:
                            emit_b(h - 1, 2 * kp + sub, apt_prev)
                        if h == 0:
                            v_chain(2 * kp + sub)
                    scores_pair(h, kp, n_dve=1 if kp in dve_kps else 0)
                if apt_prev is not None:
                    finish_b(h - 1, apt_prev)
                apt_prev = apt
            for kt in range(ST):
                emit_b(HL - 1, kt, apt_prev)
            finish_b(HL - 1, apt_prev)

    nc.compile()
    return nc


def _prep_in_maps(q, k, v, Wq, bq, Wk, bk, Wv, bv):
    qT = [np.ascontiguousarray(q[b].T.astype(BF16)) for b in range(B)]
    kT = [np.ascontiguousarray(k[b].T.astype(BF16)) for b in range(B)]
    vT = [np.ascontiguousarray(v[b].T.astype(BF16)) for b in range(B)]
    in_maps = []
    for c in range(N_CORES):
        b, hg = c // 4, c % 4
        cols = slice(hg * DH, (hg + 1) * DH)
        in_maps.append({
            "qT": qT[b],
            "kT": kT[b],
            "vT": vT[b],
            "wq": np.ascontiguousarray(Wq[:, cols].astype(BF16)),
            "wk": np.ascontiguousarray(Wk[:, cols].astype(BF16)),
            "wv": np.ascontiguousarray(Wv[:, cols].astype(BF16)),
            "bq": np.ascontiguousarray(
                bq[cols].astype(np.float32).reshape(2, 128).T),
            "bk": np.ascontiguousarray(
                bk[cols].astype(np.float32).reshape(2, 128).T),
            "bv": np.ascontiguousarray(
                np.tile(bv[cols].astype(np.float32), (128, 1))),
        })
    return in_maps


def kernel(q, k, v, Wq, bq, Wk, bk, Wv, bv, _trace=False, _trace_cores=None):
    q, k, v = (np.asarray(x, np.float32) for x in (q, k, v))
    Wq, Wk, Wv = (np.asarray(x, np.float32) for x in (Wq, Wk, Wv))
    bq, bk, bv = (np.asarray(x, np.float32) for x in (bq, bk, bv))

    if "nc" not in _CACHED:
        _CACHED["nc"] = _build()
    nc = _CACHED["nc"]

    in_maps = _prep_in_maps(q, k, v, Wq, bq, Wk, bk, Wv, bv)
    res = run_bass_kernel_spmd(
        nc, in_maps, core_ids=list(range(N_CORES)),
        trace=_trace, trace_cores=_trace_cores,
    )
    _CACHED["last_result"] = res

    out = np.empty((B, S, D), np.float32)
    for c in range(N_CORES):
        b, hg = c // 4, c % 4
        out[b, :, hg * DH:(hg + 1) * DH] = \
            res.results[c]["out"].T.astype(np.float32)
    return out


# revision 20
# speedup vs baseline: 1.4268x; 1.1908x over previous
"""Distributed Trainium2 Bass kernel for the fused attention layer.

Problem (hardcoded):
    B=2, S=2048, D=1024, H=16, HD=64.
    out = softmax((q@Wq+bq) @ (k@Wk+bk)^T / sqrt(HD)) @ (v@Wv+bv), per (b, h).

Sharding: 8 cores, core c -> batch b = c//4, head group hg = c%4 (4 heads).
Each core computes its 4 heads fully independently (no collectives) and
writes a transposed [256, S] bf16 slab; the host reassembles [B, S, D].

Per-core dataflow (matmul inputs bf16, PSUM accumulation f32):
  QhatT [2*128, S]  = (Wq_cols^T @ q_b^T) + bq   (heads on partitions)
  KhatT [2*128, S]  = (Wk_cols^T @ k_b^T) + bk
  Vaug  [S, 4*65]   = (v_b @ Wv_cols) + bv, with a ones column per head
  per head h:
    E[k, q]    = exp((KhatT_h-slice^T x QhatT_h) / 8)   (scores transposed,
                 ScalarE exp straight from PSUM, no max subtraction)
    U[65, q]   = sum_kt Vaug_h[kt]^T @ E[kt]  (row 64 = softmax denominator)
    outT[d, q] = U[0:64, q] * (1/U[64, q])    (denom broadcast via DMA)
Heads are software-pipelined: attn-V of head h-1 interleaves with
scores+exp of head h so ScalarE (the exp bottleneck) never idles.
"""

import sys
import os

for _p in ("/opt/trn_rl_repo",):
    if os.path.isdir(_p) and _p not in sys.path:
        sys.path.append(_p)

import numpy as np
import ml_dtypes

import concourse.bacc as bacc
import concourse.mybir as mybir
from concourse import tile
from concourse.bass_utils import run_bass_kernel_spmd
from concourse import dve_ops as _dvo
from concourse.dve_spec import (
    Spec as _Spec, Src0 as _S0, C0 as _C0, C1 as _C1, C2 as _C2,
    sq as _sq, lower as _lower,
)
from concourse.dve_uop import DveOpSpec as _DveOpSpec

# exp(x/8) ~= P(x/512)^64, P the cubic Taylor of e^z. Two 1-pass custom
# DVE ops (~1 elem/cycle/lane each); max rel err ~3e-4 for |x/8| <= 6.5.
_EXP_SC = 1.0 / 512.0
_EXP_ABC = (_EXP_SC ** 3 / 6.0, _EXP_SC ** 2 / 2.0, _EXP_SC)


def _register_exp_ops():
    if "EXP_POLY_ANT" in _dvo._SUB_OPCODE_FOR_NAME:
        by = {o.name: o for o in _dvo.OPS}
        return by["EXP_POLY_ANT"], by["EXP_SQ6_ANT"]

    def ref1(in0, in1, s0, s1, imm2):
        return (((in0 * s0 + s1) * in0 + imm2) * in0).astype(np.float32)

    def ref2(in0, in1, s0, s1, imm2):
        v = (in0 + s0).astype(np.float32)
        for _ in range(6):
            v = (v * v).astype(np.float32)
        return v

    body1 = ((_S0 * _C0 + _C1) * _S0 + _C2) * _S0
    body2 = _S0 + _C0
    for _ in range(6):
        body2 = _sq(body2)
    made = []
    for name, body, ref in (("EXP_POLY_ANT", body1, ref1),
                            ("EXP_SQ6_ANT", body2, ref2)):
        spec = _Spec(body=body, reference=ref)
        row = _dvo._CUSTOM_DVE_ROW_BASE + len(_dvo.OPS)
        shas = {}
        for ver in ("v3", "v4"):
            s = _DveOpSpec(name=name, opcode=row, uops=_lower(spec, ver=ver),
                           rd1_en=False)
            shas[ver] = s.sha(ver)
        op = _dvo.DveOp(name, spec, subdim=False, uops_sha=shas)
        _dvo.OPS.append(op)
        _dvo.CUSTOM_DVE_SPECS[name] = spec
        _dvo._SUB_OPCODE_FOR_NAME[name] = row
        made.append(op)
    return made


_EXP1_OP, _EXP2_OP = _register_exp_ops()


BF16 = ml_dtypes.bfloat16
N_CORES = 8
B, S, D, H = 2, 2048, 1024, 16
HD = D // H            # 64
HL = 4                 # local heads per core
DH = HL * HD           # 256 local out channels
KT = D // 128          # 8 contraction tiles for projections
ST = S // 128          # 16 sequence tiles

_CACHED = {}


def _build():
    f32 = mybir.dt.float32
    bf = mybir.dt.bfloat16
    EXP = mybir.ActivationFunctionType.Exp
    nc = bacc.Bacc(
        "TRN2", target_bir_lowering=False, debug=False, num_devices=N_CORES
    )

    qT_d = nc.dram_tensor("qT", [D, S], bf, kind="ExternalInput")
    kT_d = nc.dram_tensor("kT", [D, S], bf, kind="ExternalInput")
    vT_d = nc.dram_tensor("vT", [D, S], bf, kind="ExternalInput")
    wq_d = nc.dram_tensor("wq", [D, DH], bf, kind="ExternalInput")
    wk_d = nc.dram_tensor("wk", [D, DH], bf, kind="ExternalInput")
    wv_d = nc.dram_tensor("wv", [D, DH], bf, kind="ExternalInput")
    bq_d = nc.dram_tensor("bq", [128, 2], f32, kind="ExternalInput")
    bk_d = nc.dram_tensor("bk", [128, 2], f32, kind="ExternalInput")
    bv_d = nc.dram_tensor("bv", [128, DH], f32, kind="ExternalInput")
    out_d = nc.dram_tensor("out", [DH, S], bf, kind="ExternalOutput")

    with tile.TileContext(nc) as tc:
        with (
            tc.tile_pool(name="big", bufs=24) as big,
            tc.tile_pool(name="wp", bufs=1) as wp,
            tc.tile_pool(name="qk", bufs=1) as qk,
            tc.tile_pool(name="vp", bufs=1) as vp,
            tc.tile_pool(name="np_", bufs=1) as npl,
            tc.tile_pool(name="outp", bufs=1) as outp,
            tc.tile_pool(name="dv", bufs=2) as dvp,
            tc.tile_pool(name="ps", bufs=1, space="PSUM") as ps,
            tc.tile_pool(name="aps", bufs=1, space="PSUM") as aps,
        ):
            # ---- loads (order matters: Q-projection deps first) ----
            def load_rows(dram, n_tiles, width, pool, eng):
                ts = []
                for i in range(n_tiles):
                    t = (pool.tile([128, width], bf, tag="big",
                                   name=f"{dram.name}t{i}")
                         if pool is big else
                         pool.tile([128, width], bf, tag=f"{dram.name}{i}",
                                   name=f"{dram.name}t{i}"))
                    eng.dma_start(t[:], dram[i * 128:(i + 1) * 128, :])
                    ts.append(t)
                return ts

            warm = wp.tile([128, 512], bf, tag="warm", name="warm_t")
            nc.gpsimd.memset(warm[:], 0.0)
            for w_i in range(10):
                wps = ps.tile([128, 512], f32, tag="ps", name="warm_ps",
                              bufs=3)
                nc.tensor.matmul(wps[:], warm[:, 0:128], warm[:],
                                 start=True, stop=True)

            wq_t = load_rows(wq_d, KT, DH, wp, nc.gpsimd)
            bq_t = wp.tile([128, 2], f32, tag="bq", name="bq_t")
            nc.gpsimd.dma_start(bq_t[:], bq_d[:])
            qT_t = load_rows(qT_d, KT, S, big, nc.sync)
            wk_t = load_rows(wk_d, KT, DH, wp, nc.gpsimd)
            bk_t = wp.tile([128, 2], f32, tag="bk", name="bk_t")
            nc.gpsimd.dma_start(bk_t[:], bk_d[:])
            kT_t = load_rows(kT_d, KT, S, big, nc.sync)
            wv_t = load_rows(wv_d, KT, DH, wp, nc.gpsimd)
            bv_t = wp.tile([128, DH], f32, tag="bv", name="bv_t")
            nc.gpsimd.dma_start(bv_t[:], bv_d[:])
            vT_t = load_rows(vT_d, KT, S, big, nc.sync)

            qhat = [qk.tile([128, S], bf, tag=f"qh{mp}", name=f"qhat{mp}")
                    for mp in range(2)]
            khat = [qk.tile([128, S], bf, tag=f"kh{mp}", name=f"khat{mp}")
                    for mp in range(2)]
            vaug = [None] * ST
            texp = {}

            def qk_chain(src_t, w_t, b_t, dst, mp, nch):
                pst = ps.tile([128, 512], f32, tag="ps", name="proj_ps",
                              bufs=3)
                for kt in range(KT):
                    nc.tensor.matmul(
                        pst[:, 0:512],
                        w_t[kt][:, mp * 128:(mp + 1) * 128],
                        src_t[kt][:, nch * 512:(nch + 1) * 512],
                        start=(kt == 0), stop=(kt == KT - 1),
                    )
                nc.vector.tensor_scalar_add(
                    dst[mp][:, nch * 512:(nch + 1) * 512],
                    pst[:, 0:512], b_t[:, mp:mp + 1],
                )

            def v_chain(st):
                pst = ps.tile([128, 512], f32, tag="ps", name="vproj_ps",
                              bufs=3)
                for kt in range(KT):
                    nc.tensor.matmul(
                        pst[:, 0:DH],
                        vT_t[kt][:, st * 128:(st + 1) * 128],
                        wv_t[kt][:],
                        start=(kt == 0), stop=(kt == KT - 1),
                    )
                va = vp.tile([128, HL * 65], bf, tag=f"va{st}",
                             name=f"vaug{st}")
                nc.vector.memset(va[:], 1.0)
                for h in range(HL):
                    nc.vector.tensor_add(
                        va[:, h * 65:h * 65 + 64],
                        pst[:, h * 64:(h + 1) * 64],
                        bv_t[:, h * 64:(h + 1) * 64],
                    )
                vaug[st] = va

            def scores_pair(h, kp, n_dve=0):
                # kt = 2*kp uses khat/qhat rows [hb:hb+64] (row group A);
                # kt = 2*kp+1 uses ksh/qsh rows [64-hb:...] (row group B);
                # adjacent emission -> concurrent PE row-group execution.
                mp, hb = h // 2, (h % 2) * 64
                sb = 64 - hb
                txs = []
                for kt in (2 * kp, 2 * kp + 1):
                    txs.append(big.tile([128, S], bf, tag="big",
                                        name=f"texp_{h}_{kt}"))
                    texp[(h, kt)] = txs[-1]
                for half in range(2):
                    spts = []
                    for i, kt in enumerate((2 * kp, 2 * kp + 1)):
                        spts.append(ps.tile([128, 1024], f32, tag="ps",
                                            name=f"score_ps{i}", bufs=3))
                    for qch in range(2):
                        qlo = half * 1024 + qch * 512
                        for i, kt in enumerate((2 * kp, 2 * kp + 1)):
                            kh = khat[mp] if i == 0 else ksh[mp]
                            qh = qhat[mp] if i == 0 else qsh[mp]
                            b0 = hb if i == 0 else sb
                            nc.tensor.matmul(
                                spts[i][:, qch * 512:(qch + 1) * 512],
                                kh[b0:b0 + 64, kt * 128:(kt + 1) * 128],
                                qh[b0:b0 + 64, qlo:qlo + 512],
                                start=True, stop=True,
                            )
                    for i in range(2):
                        txs_ap = txs[i][:, half * 1024:(half + 1) * 1024]
                        if (half + i) % 2 < n_dve:
                            tmp = dvp.tile([128, 1024], f32, tag="dvtmp",
                                           name=f"dvt_{h}_{kp}_{half}_{i}")
                            a, b, c = _EXP_ABC
                            nc.vector._custom_dve(
                                _EXP1_OP, out=tmp[:], in0=spts[i][:],
                                s0=a, s1=b, imm2=c)
                            nc.vector._custom_dve(
                                _EXP2_OP, out=txs_ap, in0=tmp[:], s0=1.0)
                        else:
                            nc.scalar.activation(txs_ap, spts[i][:],
                                                 EXP, scale=0.125)

            def attnv_substep(h, ktp, c, apt):
                # chains for nch in {2c, 2c+1}; ktp advances 0..15
                for j in range(2):
                    nch = 2 * c + j
                    nc.tensor.matmul(
                        apt[j][0:65, :],
                        vaug[ktp][:, h * 65:h * 65 + 65],
                        texp[(h, ktp)][:, nch * 512:(nch + 1) * 512],
                        start=(ktp == 0), stop=(ktp == ST - 1),
                    )

            usb_t = {}

            def evac(h, tile_, nch):
                u = npl.tile([96, 512], f32, tag=f"usb{nch}",
                             name=f"usb_{h}_{nch}")
                nc.vector.tensor_copy(u[:], tile_[0:96, :])
                usb_t[(h, nch)] = u

            def finish_norm(h):
                bc = norm_t[h][2]
                ot = outp.tile([64, S], bf, tag="ot", name=f"outT{h}")
                for nch in range(4):
                    nc.vector.tensor_mul(
                        ot[:, nch * 512:(nch + 1) * 512],
                        usb_t[(h, nch)][0:64, :],
                        bc[:, nch * 512:(nch + 1) * 512],
                    )
                nc.sync.dma_start(out_d[h * 64:(h + 1) * 64, :], ot[:])

            # ---- schedule ----
            for nch in range(4):
                qk_chain(qT_t, wq_t, bq_t, qhat, 0, nch)
            for nch in range(4):
                qk_chain(kT_t, wk_t, bk_t, khat, 0, nch)
            for nch in range(4):
                qk_chain(qT_t, wq_t, bq_t, qhat, 1, nch)
            for nch in range(4):
                qk_chain(kT_t, wk_t, bk_t, khat, 1, nch)
            # half-swapped shadows: rows 0-63 <-> 64-127, so adjacent k-tiles
            # of one head can run in both PE row groups concurrently
            qsh = [qk.tile([128, S], bf, tag=f"qs{mp}", name=f"qsh{mp}")
                   for mp in range(2)]
            ksh = [qk.tile([128, S], bf, tag=f"ks{mp}", name=f"ksh{mp}")
                   for mp in range(2)]
            for mp in range(2):
                for sh, hat in ((qsh, qhat), (ksh, khat)):
                    nc.gpsimd.dma_start(sh[mp][0:64, :], hat[mp][64:128, :])
                    nc.gpsimd.dma_start(sh[mp][64:128, :], hat[mp][0:64, :])

            def alloc_apt(h, j):
                return aps.tile([96, 512], f32, tag=f"at{j % 2}",
                                name=f"attn_ps_{h}_{j}")

            norm_t = {}

            def half_norm(h, c):
                # denominator -> reciprocal -> partition broadcast for the
                # q-range covered by sweep c (nch 2c, 2c+1)
                if c == 0:
                    norm_t[h] = (
                        npl.tile([1, S], f32, tag="denrow", name=f"den{h}"),
                        npl.tile([1, S], f32, tag="rec0", name=f"rec0_{h}"),
                        npl.tile([64, S], f32, tag="bc", name=f"bc{h}"),
                    )
                denrow, rec0, bc = norm_t[h]
                for nch in (2 * c, 2 * c + 1):
                    nc.sync.dma_start(
                        denrow[0:1, nch * 512:(nch + 1) * 512],
                        usb_t[(h, nch)][64:65, :],
                    )
                sl = slice(c * 1024, (c + 1) * 1024)
                nc.vector.reciprocal_approx_fast(rec0[0:1, sl], denrow[0:1, sl])
                nc.gpsimd.partition_broadcast(bc[:, sl], rec0[0:1, sl])

            def emit_b(h, kt, apt):
                # two B-substeps per A-step: nch chains {0,1} during kt 0-7,
                # {2,3} during kt 8-15 (only 2 attn PSUM banks live at once)
                if kt == 8:
                    evac(h, apt[0], 0)
                    evac(h, apt[1], 1)
                    half_norm(h, 0)
                    apt[0], apt[1] = alloc_apt(h, 2), alloc_apt(h, 3)
                c, base = (0, 0) if kt < 8 else (1, 8)
                for j in range(2):
                    attnv_substep(h, 2 * (kt - base) + j, c, apt)

            def finish_b(h, apt):
                evac(h, apt[0], 2)
                evac(h, apt[1], 3)
                half_norm(h, 1)
                finish_norm(h)

            apt_prev = None
            for h in range(HL):
                apt = [alloc_apt(h, 0), alloc_apt(h, 1)]
                dve_kps = ({2, 5} if h == 0 else {0, 2, 4, 6})
                for kp in range(ST // 2):
                    for sub in range(2):
                        if apt_prev is not None:
                            emit_b(h - 1, 2 * kp + sub, apt_prev)
                        if h == 0:
                            v_chain(2 * kp + sub)
                    scores_pair(h, kp, n_dve=1 if kp in dve_kps else 0)
                if apt_prev is not None:
                    finish_b(h - 1, apt_prev)
                apt_prev = apt
            for kt in range(ST):
                emit_b(HL - 1, kt, apt_prev)
            finish_b(HL - 1, apt_prev)

    nc.compile()
    return nc


def _prep_in_maps(q, k, v, Wq, bq, Wk, bk, Wv, bv):
    qT = [np.ascontiguousarray(q[b].T.astype(BF16)) for b in range(B)]
    kT = [np.ascontiguousarray(k[b].T.astype(BF16)) for b in range(B)]
    vT = [np.ascontiguousarray(v[b].T.astype(BF16)) for b in range(B)]
    in_maps = []
    for c in range(N_CORES):
        b, hg = c // 4, c % 4
        cols = slice(hg * DH, (hg + 1) * DH)
        in_maps.append({
            "qT": qT[b],
            "kT": kT[b],
            "vT": vT[b],
            "wq": np.ascontiguousarray(Wq[:, cols].astype(BF16)),
            "wk": np.ascontiguousarray(Wk[:, cols].astype(BF16)),
            "wv": np.ascontiguousarray(Wv[:, cols].astype(BF16)),
            "bq": np.ascontiguousarray(
                bq[cols].astype(np.float32).reshape(2, 128).T),
            "bk": np.ascontiguousarray(
                bk[cols].astype(np.float32).reshape(2, 128).T),
            "bv": np.ascontiguousarray(
                np.tile(bv[cols].astype(np.float32), (128, 1))),
        })
    return in_maps


def kernel(q, k, v, Wq, bq, Wk, bk, Wv, bv, _trace=False, _trace_cores=None):
    q, k, v = (np.asarray(x, np.float32) for x in (q, k, v))
    Wq, Wk, Wv = (np.asarray(x, np.float32) for x in (Wq, Wk, Wv))
    bq, bk, bv = (np.asarray(x, np.float32) for x in (bq, bk, bv))

    if "nc" not in _CACHED:
        _CACHED["nc"] = _build()
    nc = _CACHED["nc"]

    in_maps = _prep_in_maps(q, k, v, Wq, bq, Wk, bk, Wv, bv)
    res = run_bass_kernel_spmd(
        nc, in_maps, core_ids=list(range(N_CORES)),
        trace=_trace, trace_cores=_trace_cores,
    )
    _CACHED["last_result"] = res

    out = np.empty((B, S, D), np.float32)
    for c in range(N_CORES):
        b, hg = c // 4, c % 4
        out[b, :, hg * DH:(hg + 1) * DH] = \
            res.results[c]["out"].T.astype(np.float32)
    return out
